# revision 22
# baseline (speedup 1.0000x reference)
"""Trainium2 Bass kernel for nn_ADSCDConv (dense_cnn), 8-core data parallel.

Per core (2 samples = 384 (b,c) channel-images of 96x96), groups of 128
partitions: g0=(b0,c0:128), g1=(b1,c0:128), g2=(b0,c128:192)||(b1,c128:192).

v2 schedule (vs v1):
  - tap-outer conv matmuls: LDWEIGHTS amortized over a multi-bank PSUM
    window; PE runs at the 202ns/FD480 streaming roofline.
  - center tap (the only theta-dependent one) is emitted LAST per window
    and deferred for the first two windows of g0, so the conv starts on
    the pooled-only dependency chain while the image-max/theta chain
    finishes.
  - windows/bandmax stats are banded (32 rows) and pipelined with the
    input DMA; x lands padded to width 100 with the payload at col 2 so
    window sums hit the DVE 4x mode.
  - DVE conv tail uses fused scalar_tensor_tensor (mul+add in one op);
    a second x copy shifted by one column (padB) keeps all taps 4B
    aligned for the 2x bf16 mode.
  - PSUM: two conv window pools (4 banks + 3 banks) alternate A,B,A,...
    globally across groups; 1 stats bank.
"""

from contextlib import ExitStack

import numpy as np
import ml_dtypes

BF16 = ml_dtypes.bfloat16

B, C, H, W = 16, 192, 96, 96
G = 4
R = C // 4  # 48
BN_EPS = 1e-5
N_CORES = 8
HP, WP = H + 2, 100  # padded rows 98, padded cols 100 (x payload at col 2)
XB_R0 = 50           # padB covers padded rows 50..97
XB_NR = 48

# conv windows: banks per window, alternating pool A(4)/B(3) globally
# (sequence across groups must alternate 4,3,4,3,... for PSUM pool reuse)
WIN_SEQ = {0: [4, 3, 4, 3], 1: [4, 3, 4, 3], 2: [4, 3, 4, 3]}
ROWS_PE = {g: 5 * sum(WIN_SEQ[g]) for g in range(3)}  # 70, 70, 70

# tap order: center (tap 4, the only theta-dependent tap) last
TAP_ORDER = [0, 1, 2, 3, 5, 6, 7, 8, 4]

_COMPILED = None


def _build():
    import concourse.tile as tile
    from concourse import bacc, mybir

    f32 = mybir.dt.float32
    bf16 = mybir.dt.bfloat16
    ALU = mybir.AluOpType
    ACTF = mybir.ActivationFunctionType

    nc = bacc.Bacc("TRN2", target_bir_lowering=False, debug=False, num_devices=N_CORES)

    # ---- DRAM tensors ----
    xA_d = nc.dram_tensor("xA", [384, HP, WP], bf16, kind="ExternalInput").ap()
    xB_d = nc.dram_tensor("xB", [384, XB_NR, WP], bf16, kind="ExternalInput").ap()
    out_d = nc.dram_tensor("out", [384, H, W], bf16, kind="ExternalOutput").ap()
    warm_d = nc.dram_tensor("warm", [128, 1], bf16, kind="ExternalOutput").ap()
    eye_d = nc.dram_tensor("eye", [128, 128], bf16, kind="ExternalInput").ap()
    w1avg_a_d = nc.dram_tensor("w1avg_a", [128, R], f32, kind="ExternalInput").ap()
    w1avg_b_d = nc.dram_tensor("w1avg_b", [128, R], f32, kind="ExternalInput").ap()
    w1mx_a_d = nc.dram_tensor("w1mx_a", [128, R], f32, kind="ExternalInput").ap()
    w1mx_b_d = nc.dram_tensor("w1mx_b", [128, R], f32, kind="ExternalInput").ap()
    w2t_d = nc.dram_tensor("w2t", [R, C], f32, kind="ExternalInput").ap()
    p1a_d = nc.dram_tensor("p1a", [128, R], f32, kind="ExternalInput").ap()
    p1b_d = nc.dram_tensor("p1b", [128, R], f32, kind="ExternalInput").ap()
    bns_d = nc.dram_tensor("bn_scale", [R, 1], f32, kind="ExternalInput").ap()
    bnb_d = nc.dram_tensor("bn_beta", [R, 1], f32, kind="ExternalInput").ap()
    w2s_d = nc.dram_tensor("w2s", [R, G * C], f32, kind="ExternalInput").ap()
    adkT_d = nc.dram_tensor("adkT", [384, 36], f32, kind="ExternalInput").ap()

    with tile.TileContext(nc) as tc, ExitStack() as ctx:
        def sb(name, shape, dt):
            return nc.alloc_sbuf_tensor(name, shape, dt).ap()

        padA = [sb(f"padA{g}", [128, HP, WP], bf16) for g in range(3)]
        padB = [sb(f"padB{g}", [128, XB_NR, WP], bf16) for g in range(3)]
        padAf = [p.rearrange("p a b -> p (a b)") for p in padA]
        padBf = [p.rearrange("p a b -> p (a b)") for p in padB]
        tailb = [sb(f"tail{g}", [128, 31, W], bf16) for g in range(3)]
        diag = [sb(f"diag{g}", [128, 9, 128], bf16) for g in range(3)]
        pooled = [sb(f"pooled{g}", [128, 9], f32) for g in range(3)]
        avgs = [sb(f"avgs{g}", [128, 1], f32) for g in range(3)]
        mx = [sb(f"mx{g}", [128, 1], f32) for g in range(3)]
        th = [sb(f"theta{g}", [128, 1], f32) for g in range(3)]
        w9 = [sb(f"w9_{g}", [128, 9], f32) for g in range(3)]
        w4p = [sb(f"w4p{g}", [128, 1], f32) for g in range(3)]
        wsum9 = [sb(f"wsum9_{g}", [128, 1], f32) for g in range(3)]
        adkT = [sb(f"adkT{g}_sb", [128, 36], f32) for g in range(3)]

        eye = sb("eye_sb", [128, 128], bf16)
        w1avg_a = sb("w1avg_a_sb", [128, R], f32)
        w1avg_b = sb("w1avg_b_sb", [128, R], f32)
        w1mx_a = sb("w1mx_a_sb", [128, R], f32)
        w1mx_b = sb("w1mx_b_sb", [128, R], f32)
        w2t = sb("w2t_sb", [R, C], f32)
        p1a = sb("p1a_sb", [128, R], f32)
        p1b = sb("p1b_sb", [128, R], f32)
        bns = sb("bns_sb", [R, 1], f32)
        bnb = sb("bnb_sb", [R, 1], f32)
        w2s = sb("w2s_sb", [R, G * C], f32)

        h_adk = [sb(f"h_adk{b}", [R, 9], f32) for b in range(2)]
        hsum = [sb(f"hsum{b}", [R, 1], f32) for b in range(2)]

        scr = ctx.enter_context(tc.tile_pool(name="scr", bufs=4))
        treep = ctx.enter_context(tc.tile_pool(name="treep", bufs=2))
        term_pool = ctx.enter_context(tc.tile_pool(name="terms", bufs=3))
        osb_pool = ctx.enter_context(tc.tile_pool(name="osbp", bufs=8))
        ct_pool = ctx.enter_context(tc.tile_pool(name="ctp", bufs=3))
        psA = ctx.enter_context(tc.tile_pool(name="psA", bufs=1, space="PSUM"))
        psB = ctx.enter_context(tc.tile_pool(name="psB", bufs=1, space="PSUM"))
        # stats bank: single-shot matmul groups only may share a bank
        stpa = nc.alloc_psum_tensor("statps", [128, 512], f32).ap()

        # ---------------- DMA emission ----------------
        row_chunks = [(0, 33), (33, 65), (65, HP)]

        def emit_xA_dma(g):
            for (r0, r1) in row_chunks:
                nc.sync.dma_start(
                    out=padA[g][:, r0:r1, :],
                    in_=xA_d[g * 128:(g + 1) * 128, r0:r1, :],
                )

        def emit_xB_dma(g):
            # WAW gate: tiny DVE write into padB dependent on g1's last xA
            # chunk keeps the xB transfer out of the critical input window
            nc.vector.tensor_copy(padB[g][:, 0:1, 0:2], padA[1][:, 97:98, 0:2])
            nc.scalar.dma_start(
                out=padB[g][:, :, :],
                in_=xB_d[g * 128:(g + 1) * 128, :, :],
            )

        emit_xA_dma(0)
        wloads = [
            (eye, eye_d), (w1avg_a, w1avg_a_d), (w1avg_b, w1avg_b_d),
            (w1mx_a, w1mx_a_d), (w1mx_b, w1mx_b_d), (w2t, w2t_d),
            (p1a, p1a_d), (p1b, p1b_d), (bns, bns_d), (bnb, bnb_d),
            (w2s, w2s_d),
            (adkT[0], adkT_d[0:128, :]), (adkT[1], adkT_d[128:256, :]),
            (adkT[2], adkT_d[256:384, :]),
        ]
        for (dst, src) in wloads:
            nc.gpsimd.dma_start(out=dst, in_=src)
        emit_xA_dma(2)
        emit_xA_dma(1)

        # ---------------- stats ----------------
        def emit_band_windows(g, k, engine="vector"):
            # 3 col-window sums of the 32-row band k -> pooled[g][:, 3k+j]
            if engine == "scalar":
                for j in range(3):
                    win = padA[g][:, 1 + 32 * k:33 + 32 * k, 2 + 32 * j:34 + 32 * j]
                    acc = pooled[g][:, 3 * k + j:3 * k + j + 1]
                    s = treep.tile([128, 32, 32], bf16, tag="wscr", name=f"w{g}_{k}_{j}")
                    nc.scalar.activation(out=s[:, :, :], in_=win,
                                         func=ACTF.Copy, accum_out=acc)
                return
            # DVE: bf16 TT add-tree 32->16->8->4 rows (2x mode), then 3
            # cache-reduce window sums over the 4 leaf rows
            p = padA[g]
            r0 = 1 + 32 * k
            t16 = treep.tile([128, 16, WP], bf16, tag="tr16", name=f"s16_{g}_{k}")
            nc.vector.tensor_add(t16[:, :, :], p[:, r0:r0 + 16, :], p[:, r0 + 16:r0 + 32, :])
            t8 = treep.tile([128, 8, WP], bf16, tag="tr8", name=f"s8_{g}_{k}")
            nc.vector.tensor_add(t8[:, :, :], t16[:, 0:8, :], t16[:, 8:16, :])
            t4 = treep.tile([128, 4, WP], bf16, tag="tr4", name=f"s4_{g}_{k}")
            nc.vector.tensor_add(t4[:, :, :], t8[:, 0:4, :], t8[:, 4:8, :])
            for j in range(3):
                acc = pooled[g][:, 3 * k + j:3 * k + j + 1]
                s = treep.tile([128, 4, 32], bf16, tag="wscr4", name=f"w{g}_{k}_{j}")
                nc.vector.tensor_scalar(s[:, :, :],
                                        t4[:, :, 2 + 32 * j:34 + 32 * j],
                                        1.0, None,
                                        op0=ALU.mult, op1=ALU.add, accum_out=acc)

        lvmax = [sb(f"lvmax{g}", [128, 12, WP], bf16) for g in range(3)]

        def emit_band_max(g, k):
            # bf16 TT max-tree 32->16->8->4 rows into lvmax[g][:, 4k:4k+4]
            p = padA[g]
            r0 = 1 + 32 * k
            t16 = treep.tile([128, 16, WP], bf16, tag="tr16", name=f"m16_{g}_{k}")
            nc.vector.tensor_tensor(out=t16[:, :, :], in0=p[:, r0:r0 + 16, :],
                                    in1=p[:, r0 + 16:r0 + 32, :], op=ALU.max)
            t8 = treep.tile([128, 8, WP], bf16, tag="tr8", name=f"m8_{g}_{k}")
            nc.vector.tensor_tensor(out=t8[:, :, :], in0=t16[:, 0:8, :],
                                    in1=t16[:, 8:16, :], op=ALU.max)
            nc.vector.tensor_tensor(out=lvmax[g][:, 4 * k:4 * k + 4, :],
                                    in0=t8[:, 0:4, :], in1=t8[:, 4:8, :], op=ALU.max)

        def emit_mx_fin(g):
            t6 = treep.tile([128, 6, WP], bf16, tag="tr6", name=f"mf6_{g}")
            nc.vector.tensor_tensor(out=t6[:, :, :], in0=lvmax[g][:, 0:6, :],
                                    in1=lvmax[g][:, 6:12, :], op=ALU.max)
            t3 = treep.tile([128, 3, WP], bf16, tag="tr3", name=f"mf3_{g}")
            nc.vector.tensor_tensor(out=t3[:, :, :], in0=t6[:, 0:3, :],
                                    in1=t6[:, 3:6, :], op=ALU.max)
            nc.vector.tensor_reduce(out=mx[g][:, :], in_=t3[:, :, :],
                                    axis=mybir.AxisListType.XY, op=ALU.max)

        def emit_avg_fin(g):
            asc = scr.tile([128, 9], bf16, tag="ascr", name=f"avg{g}")
            nc.scalar.activation(out=asc[:, :], in_=pooled[g][:, :],
                                 func=ACTF.Copy, accum_out=avgs[g][:, :])

        # ---------------- per-sample algebra ----------------
        def emit_sample(b, part):
            if b == 0:
                chunks = [
                    (w1avg_a[:, :], w1mx_a[:, :], p1a[:, :], (0, 0, 128)),
                    (w1avg_b[0:64, :], w1mx_b[0:64, :], p1b[0:64, :], (2, 0, 64)),
                ]
            else:
                chunks = [
                    (w1avg_a[:, :], w1mx_a[:, :], p1a[:, :], (1, 0, 128)),
                    (w1avg_b[64:128, :], w1mx_b[64:128, :], p1b[64:128, :], (2, 64, 128)),
                ]
            base = 8 + b * 22
            for i, (wa, wm, wp, (sg, q0, q1)) in enumerate(chunks):
                o = base + 11 * i
                if part == "pool":
                    nc.tensor.matmul(stpa[0:R, o + 2:o + 11], lhsT=wp, rhs=pooled[sg][q0:q1, :], start=True, stop=True)
                else:
                    nc.tensor.matmul(stpa[0:R, o:o + 1], lhsT=wa, rhs=avgs[sg][q0:q1, :], start=True, stop=True)
                    nc.tensor.matmul(stpa[0:R, o + 1:o + 2], lhsT=wm, rhs=mx[sg][q0:q1, :], start=True, stop=True)

        def emit_fold_pool(b):
            base = 8 + b * 22
            hc = scr.tile([R, 9], f32, tag="scr48", name=f"hc{b}")
            nc.vector.tensor_copy(hc[:, :], stpa[0:R, base + 2:base + 11])
            hs = scr.tile([R, 9], f32, tag="scr48", name=f"hs{b}")
            nc.vector.tensor_add(hs[:, :], hc[:, :],
                                 stpa[0:R, base + 13:base + 22])
            t1 = scr.tile([R, 9], f32, tag="scr48", name=f"bn{b}")
            nc.vector.tensor_scalar(t1[:, :], hs[:, :], bns[:, :], bnb[:, :],
                                    op0=ALU.mult, op1=ALU.add)
            nc.vector.tensor_scalar_max(h_adk[b][:, :], t1[:, :], 0.0)

        def emit_fold_theta(b):
            base = 8 + b * 22
            hg = scr.tile([R, 2], f32, tag="scr2", name=f"hg{b}")
            nc.vector.tensor_copy(hg[:, :], stpa[0:R, base:base + 2])
            hs = scr.tile([R, 2], f32, tag="scr2", name=f"ht{b}")
            nc.vector.tensor_add(hs[:, :], hg[:, :],
                                 stpa[0:R, base + 11:base + 13])
            ha = scr.tile([R, 1], f32, tag="scr1", name=f"ha{b}")
            hm = scr.tile([R, 1], f32, tag="scr1", name=f"hm{b}")
            nc.vector.tensor_scalar_max(ha[:, :], hs[:, 0:1], 0.0)
            nc.vector.tensor_scalar_max(hm[:, :], hs[:, 1:2], 0.0)
            nc.vector.tensor_add(hsum[b][:, :], ha[:, :], hm[:, :])

        # ---------------- theta ----------------
        ps_t = [stpa[:, i:i + 1] for i in range(3)]

        def emit_theta_mm(b):
            nc.tensor.matmul(ps_t[b], lhsT=w2t[:, 0:128], rhs=hsum[b][:, :], start=True, stop=True)
            q0, q1 = (0, 64) if b == 0 else (64, 128)
            nc.tensor.matmul(ps_t[2][q0:q1], lhsT=w2t[:, 128:192], rhs=hsum[b][:, :], start=True, stop=True)

        def emit_theta_fin(g):
            et = scr.tile([128, 1], f32, tag="scr1", name=f"et{g}")
            nc.scalar.activation(out=et[:, :], in_=ps_t[g], func=ACTF.Exp, scale=-1.0)
            d = scr.tile([128, 1], f32, tag="scr1", name=f"etd{g}")
            nc.vector.tensor_scalar_add(d[:, :], et[:, :], 1.0)
            nc.vector.reciprocal(th[g][:, :], d[:, :])

        # ---------------- dynamic kernels w9 ----------------
        def emit_w9_mm(g):
            ps_s = stpa[:, 64 + g * 36:64 + (g + 1) * 36]
            for gg in range(G):
                sl = slice(gg * 9, gg * 9 + 9)
                if g < 2:
                    nc.tensor.matmul(ps_s[:, sl], lhsT=w2s[:, gg * 192:gg * 192 + 128],
                                     rhs=h_adk[g][:, :], start=True, stop=True)
                else:
                    nc.tensor.matmul(ps_s[0:64, sl], lhsT=w2s[:, gg * 192 + 128:gg * 192 + 192],
                                     rhs=h_adk[0][:, :], start=True, stop=True)
                    nc.tensor.matmul(ps_s[64:128, sl], lhsT=w2s[:, gg * 192 + 128:gg * 192 + 192],
                                     rhs=h_adk[1][:, :], start=True, stop=True)

        def emit_w9_exp(g):
            ps_s = stpa[:, 64 + g * 36:64 + (g + 1) * 36]
            e = scr.tile([128, 36], f32, tag="scr36", name=f"e{g}")
            nc.scalar.activation(out=e[:, :], in_=ps_s, func=ACTF.Exp)
            return e

        def emit_w9_fin(g, e):
            d1 = scr.tile([128, 9], f32, tag="scr9", name=f"d1_{g}")
            d2 = scr.tile([128, 9], f32, tag="scr9", name=f"d2_{g}")
            nc.vector.tensor_add(d1[:, :], e[:, 0:9], e[:, 9:18])
            nc.vector.tensor_add(d2[:, :], e[:, 18:27], e[:, 27:36])
            nc.vector.tensor_add(d1[:, :], d1[:, :], d2[:, :])
            rec = scr.tile([128, 9], f32, tag="scr9", name=f"rec{g}")
            nc.vector.reciprocal(rec[:, :], d1[:, :])
            a = adkT[g]
            m1 = scr.tile([128, 9], f32, tag="scr9", name=f"m1_{g}")
            m2 = scr.tile([128, 9], f32, tag="scr9", name=f"m2_{g}")
            nc.vector.tensor_mul(m1[:, :], e[:, 0:9], a[:, 0:9])
            nc.vector.tensor_mul(m2[:, :], e[:, 9:18], a[:, 9:18])
            nc.vector.tensor_add(m1[:, :], m1[:, :], m2[:, :])
            nc.vector.tensor_mul(m2[:, :], e[:, 18:27], a[:, 18:27])
            nc.vector.tensor_add(m1[:, :], m1[:, :], m2[:, :])
            nc.vector.tensor_mul(m2[:, :], e[:, 27:36], a[:, 27:36])
            nc.vector.tensor_add(m1[:, :], m1[:, :], m2[:, :])
            nc.vector.tensor_mul(w9[g][:, :], m1[:, :], rec[:, :])
            nc.vector.tensor_reduce(out=wsum9[g][:, :], in_=w9[g][:, :],
                                    axis=mybir.AxisListType.X, op=ALU.add)

        def emit_w4p(g):
            t1 = scr.tile([128, 1], f32, tag="scr1", name=f"t1_{g}")
            nc.vector.tensor_mul(t1[:, :], w9[g][:, 4:5], th[g][:, :])
            nc.vector.tensor_add(t1[:, :], t1[:, :], w9[g][:, 4:5])
            nc.vector.tensor_sub(w4p[g][:, :], t1[:, :], wsum9[g][:, :])

        def emit_diag(g, taps, engine):
            for tap in taps:
                scal = w4p[g][:, 0:1] if tap == 4 else w9[g][:, tap:tap + 1]
                if engine == "vector":
                    nc.vector.tensor_scalar_mul(diag[g][:, tap, :], eye[:, :], scal)
                else:
                    nc.scalar.activation(out=diag[g][:, tap, :], in_=eye[:, :],
                                         func=ACTF.Copy, scale=scal)

        # ---------------- conv on PE ----------------
        # window w of group g covers rows win_r0 .. win_r0+5*banks
        win_r0 = {}
        for g in range(3):
            r = 0
            for w, nb in enumerate(WIN_SEQ[g]):
                win_r0[(g, w)] = r
                r += 5 * nb

        pools = {4: psA, 3: psB}
        win_tile = {}
        out_rr = [0]

        def conv_taps(g, w, taps):
            nb = WIN_SEQ[g][w]
            r0 = win_r0[(g, w)]
            key = (g, w)
            if key not in win_tile:
                pnb = 4 if w % 2 == 0 else 3
                win_tile[key] = pools[pnb].tile(
                    [128, pnb, 512], f32, tag=f"w{pnb}", name=f"ps{g}_{w}")
            ps = win_tile[key]
            for tap in taps:
                dy, dx = divmod(tap, 3)
                for b in range(nb):
                    y0 = r0 + 5 * b + dy
                    nc.tensor.matmul(
                        ps[:, b, 0:480],
                        lhsT=diag[g][:, tap, :],
                        rhs=padA[g][:, y0:y0 + 5, dx + 1:dx + 97],
                        start=(tap == taps[0]), stop=(tap == taps[-1]),
                    )

        def conv_drain(g, w):
            nb = WIN_SEQ[g][w]
            r0 = win_r0[(g, w)]
            ps = win_tile.pop((g, w))
            nr = 5 * nb
            ot = osb_pool.tile([128, 20, W], bf16, tag="ow", name=f"ow{g}_{w}")
            nc.scalar.activation(
                out=ot[:, 0:nr, :],
                in_=ps[:, 0:nb, 0:480], func=ACTF.Copy)
            eng = (nc.sync, nc.gpsimd)[out_rr[0] % 2]
            out_rr[0] += 1
            eng.dma_start(
                out=out_d[g * 128:(g + 1) * 128, r0:r0 + nr, :],
                in_=ot[:, 0:nr, :])

        # ---------------- conv tail on DVE ----------------
        def emit_conv_dve(g, lo, hi):
            # DVE tail rows: flat contiguous strips (full padded rows, junk
            # at pad columns) so the muls hit the 4x DVE mode; only the
            # final add back into tailb is strided.
            y0 = ROWS_PE[g] + lo
            n = hi - lo
            L = (n - 1) * 100 + 96
            acc = None
            for i, tap in enumerate(TAP_ORDER):
                dy, dx = divmod(tap, 3)
                scal = w4p[g][:, 0:1] if tap == 4 else w9[g][:, tap:tap + 1]
                if dx == 1:
                    o0 = (y0 + dy) * 100 + 2
                    strip = padAf[g][:, o0:o0 + L]
                else:
                    o0 = (y0 + dy - XB_R0) * 100 + (2 if dx == 0 else 4)
                    strip = padBf[g][:, o0:o0 + L]
                t = term_pool.tile([128, 14, 100], bf16, tag="term",
                                   name=f"t{g}_{lo}_{i}")
                tf = t.rearrange("p a b -> p (a b)")
                nc.vector.tensor_scalar_mul(tf[:, 0:L], strip, scal)
                if i == 0:
                    acc = t
                elif i < 8:
                    nxt = term_pool.tile([128, 14, 100], bf16, tag="term",
                                         name=f"a{g}_{lo}_{i}")
                    nxf = nxt.rearrange("p a b -> p (a b)")
                    nc.vector.tensor_add(nxf[:, 0:L],
                                         acc.rearrange("p a b -> p (a b)")[:, 0:L],
                                         tf[:, 0:L])
                    acc = nxt
                else:
                    nc.vector.tensor_add(tailb[g][:, lo:hi, :],
                                         acc[:, 0:n, 0:96], t[:, 0:n, 0:96])
            eng = (nc.sync, nc.gpsimd)[out_rr[0] % 2]
            out_rr[0] += 1
            eng.dma_start(out=out_d[g * 128:(g + 1) * 128, y0:y0 + n, :],
                          in_=tailb[g][:, lo:hi, :])

        # ---------------- PE warm-up ----------------
        # junk matmuls gated on successive DMA chunks so the PE stays busy
        # (HAM warm) across the whole stats prelude without running eagerly
        def emit_warmup(k, gate, drain=False):
            for j in range(k):
                nc.tensor.matmul(stpa[:, 384:512], lhsT=eye[:, :],
                                 rhs=gate, start=True, stop=True)
            if drain:
                wsc = scr.tile([128, 1], bf16, tag="wscr1", name="wscr")
                nc.scalar.activation(out=wsc[:, :], in_=stpa[:, 384:385], func=ACTF.Copy)
                nc.sync.dma_start(out=warm_d, in_=wsc[:, :])

        # ---------------- emission order ----------------
        NC8 = TAP_ORDER[:8]

        # prelude: window sums first (pooled -> w9 -> conv is the critical
        # path; image-max/theta only gates the post-drain center-tap add)
        for k in range(3):
            emit_band_windows(0, k, engine="vector")
        emit_band_windows(2, 0, engine="scalar")
        emit_band_windows(2, 1, engine="scalar")
        emit_band_windows(2, 2, engine="vector")
        emit_avg_fin(0)
        emit_avg_fin(2)
        emit_xB_dma(0)
        emit_xB_dma(1)
        emit_xB_dma(2)
        emit_warmup(30, padA[0][:, 65:67, 0:64])
        emit_warmup(30, padA[2][:, 65:67, 0:64])
        emit_sample(0, "pool")
        emit_fold_pool(0)
        emit_w9_mm(0)
        e0 = emit_w9_exp(0)
        emit_w9_fin(0, e0)
        emit_diag(0, NC8, "vector")
        for k in range(3):
            emit_band_max(0, k)
            emit_band_max(2, k)
        emit_mx_fin(0)
        emit_mx_fin(2)
        emit_warmup(24, padA[2][:, 10:12, 32:96])
        emit_warmup(16, padA[2][:, 40:42, 32:96], drain=True)
        emit_sample(0, "theta")
        emit_fold_theta(0)
        emit_theta_mm(0)
        emit_theta_fin(0)
        emit_w4p(0)
        emit_diag(0, [4], "vector")

        conv_taps(0, 0, TAP_ORDER)
        conv_taps(0, 1, TAP_ORDER)

        # g1 stats (data lands mid-conv-g0); sums on ScalarE (slack there)
        for k in range(3):
            emit_band_windows(1, k, engine="scalar")
            emit_band_max(1, k)
        emit_mx_fin(1)
        emit_avg_fin(1)

        conv_drain(0, 0)
        conv_taps(0, 2, TAP_ORDER)
        emit_sample(1, "pool")
        emit_fold_pool(1)
        emit_sample(1, "theta")
        emit_fold_theta(1)
        emit_theta_mm(1)
        emit_theta_fin(1)
        emit_w9_mm(1)
        e1 = emit_w9_exp(1)
        emit_w9_fin(1, e1)
        emit_w4p(1)
        emit_diag(1, TAP_ORDER, "scalar")
        conv_drain(0, 1)
        conv_taps(0, 3, TAP_ORDER)
        emit_theta_fin(2)
        emit_w9_mm(2)
        e2 = emit_w9_exp(2)
        emit_w9_fin(2, e2)
        emit_w4p(2)
        emit_diag(2, TAP_ORDER, "scalar")
        conv_drain(0, 2)
        conv_taps(1, 0, TAP_ORDER)
        conv_drain(0, 3)
        emit_conv_dve(0, 0, 13)
        conv_taps(1, 1, TAP_ORDER)
        conv_drain(1, 0)
        emit_conv_dve(0, 13, 26)
        conv_taps(1, 2, TAP_ORDER)
        conv_drain(1, 1)
        emit_conv_dve(1, 0, 13)
        conv_taps(1, 3, TAP_ORDER)
        conv_drain(1, 2)
        emit_conv_dve(1, 13, 26)
        conv_taps(2, 0, TAP_ORDER)
        conv_drain(1, 3)
        conv_taps(2, 1, TAP_ORDER)
        conv_drain(2, 0)
        emit_conv_dve(2, 0, 13)
        conv_taps(2, 2, TAP_ORDER)
        conv_drain(2, 1)
        emit_conv_dve(2, 13, 26)
        conv_taps(2, 3, TAP_ORDER)
        conv_drain(2, 2)
        conv_drain(2, 3)

    nc.compile()
    return nc


def _host_prep(inputs):
    x = np.ascontiguousarray(inputs["x"], dtype=np.float32)
    cam_w1 = np.asarray(inputs["cam_w1"], dtype=np.float32)
    cam_w2 = np.asarray(inputs["cam_w2"], dtype=np.float32)
    proj_w1 = np.asarray(inputs["proj_w1"], dtype=np.float32)
    bn_gamma = np.asarray(inputs["bn_gamma"], dtype=np.float32)
    bn_beta = np.asarray(inputs["bn_beta"], dtype=np.float32)
    proj_w2 = np.asarray(inputs["proj_w2"], dtype=np.float32)
    adk = np.asarray(inputs["adk_weight"], dtype=np.float32)

    xb16 = x.astype(BF16)
    xpA = np.zeros((B, C, HP, WP), dtype=BF16)
    xpA[:, :, 1:97, 2:98] = xb16
    # padB: x payload at col 3, rows = padded rows 50..97 (x rows 49..95)
    xpB = np.zeros((B, C, XB_NR, WP), dtype=BF16)
    xpB[:, :, 0:47, 3:99] = xb16[:, :, 49:96, :]

    in_maps = []
    w1t = cam_w1.T.astype(np.float32)
    p1t = (proj_w1.T / 1024.0).astype(np.float32)
    cmap = np.concatenate([np.arange(128), np.arange(128),
                           np.arange(128, 192), np.arange(128, 192)])
    consts = {
        "eye": np.eye(128, dtype=BF16),
        "w1avg_a": np.ascontiguousarray(w1t[0:128] / (H * W)),
        "w1avg_b": np.ascontiguousarray(np.concatenate([w1t[128:192] / (H * W)] * 2, axis=0)),
        "w1mx_a": np.ascontiguousarray(w1t[0:128]),
        "w1mx_b": np.ascontiguousarray(np.concatenate([w1t[128:192]] * 2, axis=0)),
        "w2t": np.ascontiguousarray(cam_w2.T.astype(np.float32)),
        "p1a": np.ascontiguousarray(p1t[0:128]),
        "p1b": np.ascontiguousarray(np.concatenate([p1t[128:192]] * 2, axis=0)),
        "bn_scale": np.ascontiguousarray((bn_gamma / np.sqrt(1.0 + BN_EPS)).reshape(R, 1)),
        "bn_beta": np.ascontiguousarray(bn_beta.reshape(R, 1)),
        "w2s": np.ascontiguousarray(proj_w2.T.astype(np.float32)),
        "adkT": np.ascontiguousarray(
            adk.transpose(1, 0, 2, 3).reshape(C, G * 9)[cmap].astype(np.float32)
        ),
    }
    for k in range(N_CORES):
        b0, b1 = 2 * k, 2 * k + 1
        shardA = np.ascontiguousarray(np.concatenate(
            [xpA[b0, 0:128], xpA[b1, 0:128], xpA[b0, 128:192], xpA[b1, 128:192]],
            axis=0))
        shardB = np.ascontiguousarray(np.concatenate(
            [xpB[b0, 0:128], xpB[b1, 0:128], xpB[b0, 128:192], xpB[b1, 128:192]],
            axis=0))
        m = {"xA": shardA, "xB": shardB}
        m.update(consts)
        in_maps.append(m)
    return in_maps


def kernel(**inputs) -> np.ndarray:
    global _COMPILED
    from concourse.bass_utils import run_bass_kernel_spmd

    in_maps = _host_prep(inputs)

    if _COMPILED is None:
        _COMPILED = _build()
    nc = _COMPILED

    res = run_bass_kernel_spmd(nc, in_maps, core_ids=list(range(N_CORES)))
    outs = [r["out"] for r in res.results]

    y = np.empty((B, C, H, W), np.float32)
    for k in range(N_CORES):
        o = np.asarray(outs[k]).reshape(384, H, W).astype(np.float32)
        b0, b1 = 2 * k, 2 * k + 1
        y[b0, 0:128] = o[0:128]
        y[b1, 0:128] = o[128:256]
        y[b0, 128:192] = o[256:320]
        y[b1, 128:192] = o[320:384]
    return y


if __name__ == "__main__":
    import reference

    inputs = {k: np.asarray(v) for k, v in reference.setup_inputs().items()}
    y = kernel(**inputs)
    print("kernel output:", y.shape, y.dtype)



# revision 23
# speedup vs baseline: 1.0147x; 1.0147x over previous
"""Trainium2 Bass kernel for nn_ADSCDConv (dense_cnn), 8-core data parallel.

Per core (2 samples = 384 (b,c) channel-images of 96x96), groups of 128
partitions: g0=(b0,c0:128), g1=(b1,c0:128), g2=(b0,c128:192)||(b1,c128:192).

v2 schedule (vs v1):
  - tap-outer conv matmuls: LDWEIGHTS amortized over a multi-bank PSUM
    window; PE runs at the 202ns/FD480 streaming roofline.
  - center tap (the only theta-dependent one) is emitted LAST per window
    and deferred for the first two windows of g0, so the conv starts on
    the pooled-only dependency chain while the image-max/theta chain
    finishes.
  - windows/bandmax stats are banded (32 rows) and pipelined with the
    input DMA; x lands padded to width 100 with the payload at col 2 so
    window sums hit the DVE 4x mode.
  - DVE conv tail uses fused scalar_tensor_tensor (mul+add in one op);
    a second x copy shifted by one column (padB) keeps all taps 4B
    aligned for the 2x bf16 mode.
  - PSUM: two conv window pools (4 banks + 3 banks) alternate A,B,A,...
    globally across groups; 1 stats bank.
"""

from contextlib import ExitStack

import numpy as np
import ml_dtypes

BF16 = ml_dtypes.bfloat16

B, C, H, W = 16, 192, 96, 96
G = 4
R = C // 4  # 48
BN_EPS = 1e-5
N_CORES = 8
HP, WP = H + 2, 100  # padded rows 98, padded cols 100 (x payload at col 2)
XB_R0 = 50           # padB covers padded rows 50..97
XB_NR = 48

# conv windows: banks per window, alternating pool A(4)/B(3) globally
# (sequence across groups must alternate 4,3,4,3,... for PSUM pool reuse)
WIN_SEQ = {0: [4, 3, 4, 3], 1: [4, 3, 4, 2], 2: [4, 3, 4, 3]}
ROWS_PE = {g: 5 * sum(WIN_SEQ[g]) for g in range(3)}  # 70, 65, 70

# tap order: center (tap 4, the only theta-dependent tap) last
TAP_ORDER = [0, 1, 2, 3, 5, 6, 7, 8, 4]

_COMPILED = None


def _build():
    import concourse.tile as tile
    from concourse import bacc, mybir

    f32 = mybir.dt.float32
    bf16 = mybir.dt.bfloat16
    ALU = mybir.AluOpType
    ACTF = mybir.ActivationFunctionType

    nc = bacc.Bacc("TRN2", target_bir_lowering=False, debug=False, num_devices=N_CORES)

    # ---- DRAM tensors ----
    xA_d = nc.dram_tensor("xA", [384, HP, WP], bf16, kind="ExternalInput").ap()
    xB_d = nc.dram_tensor("xB", [384, XB_NR, WP], bf16, kind="ExternalInput").ap()
    out_d = nc.dram_tensor("out", [384, H, W], bf16, kind="ExternalOutput").ap()
    warm_d = nc.dram_tensor("warm", [128, 1], bf16, kind="ExternalOutput").ap()
    eye_d = nc.dram_tensor("eye", [128, 128], bf16, kind="ExternalInput").ap()
    w1avg_a_d = nc.dram_tensor("w1avg_a", [128, R], f32, kind="ExternalInput").ap()
    w1avg_b_d = nc.dram_tensor("w1avg_b", [128, R], f32, kind="ExternalInput").ap()
    w1mx_a_d = nc.dram_tensor("w1mx_a", [128, R], f32, kind="ExternalInput").ap()
    w1mx_b_d = nc.dram_tensor("w1mx_b", [128, R], f32, kind="ExternalInput").ap()
    w2t_d = nc.dram_tensor("w2t", [R, C], f32, kind="ExternalInput").ap()
    p1a_d = nc.dram_tensor("p1a", [128, R], f32, kind="ExternalInput").ap()
    p1b_d = nc.dram_tensor("p1b", [128, R], f32, kind="ExternalInput").ap()
    bns_d = nc.dram_tensor("bn_scale", [R, 1], f32, kind="ExternalInput").ap()
    bnb_d = nc.dram_tensor("bn_beta", [R, 1], f32, kind="ExternalInput").ap()
    w2s_d = nc.dram_tensor("w2s", [R, G * C], f32, kind="ExternalInput").ap()
    adkT_d = nc.dram_tensor("adkT", [384, 36], f32, kind="ExternalInput").ap()

    with tile.TileContext(nc) as tc, ExitStack() as ctx:
        def sb(name, shape, dt):
            return nc.alloc_sbuf_tensor(name, shape, dt).ap()

        padA = [sb(f"padA{g}", [128, HP, WP], bf16) for g in range(3)]
        padB = [sb(f"padB{g}", [128, XB_NR, WP], bf16) for g in range(3)]
        padAf = [p.rearrange("p a b -> p (a b)") for p in padA]
        padBf = [p.rearrange("p a b -> p (a b)") for p in padB]
        tailb = [sb(f"tail{g}", [128, 31, W], bf16) for g in range(3)]
        diag = [sb(f"diag{g}", [128, 9, 128], bf16) for g in range(3)]
        pooled = [sb(f"pooled{g}", [128, 9], f32) for g in range(3)]
        avgs = [sb(f"avgs{g}", [128, 1], f32) for g in range(3)]
        mx = [sb(f"mx{g}", [128, 1], f32) for g in range(3)]
        th = [sb(f"theta{g}", [128, 1], f32) for g in range(3)]
        w9 = [sb(f"w9_{g}", [128, 9], f32) for g in range(3)]
        w4p = [sb(f"w4p{g}", [128, 1], f32) for g in range(3)]
        wsum9 = [sb(f"wsum9_{g}", [128, 1], f32) for g in range(3)]
        adkT = [sb(f"adkT{g}_sb", [128, 36], f32) for g in range(3)]

        eye = sb("eye_sb", [128, 128], bf16)
        w1avg_a = sb("w1avg_a_sb", [128, R], f32)
        w1avg_b = sb("w1avg_b_sb", [128, R], f32)
        w1mx_a = sb("w1mx_a_sb", [128, R], f32)
        w1mx_b = sb("w1mx_b_sb", [128, R], f32)
        w2t = sb("w2t_sb", [R, C], f32)
        p1a = sb("p1a_sb", [128, R], f32)
        p1b = sb("p1b_sb", [128, R], f32)
        bns = sb("bns_sb", [R, 1], f32)
        bnb = sb("bnb_sb", [R, 1], f32)
        w2s = sb("w2s_sb", [R, G * C], f32)

        h_adk = [sb(f"h_adk{b}", [R, 9], f32) for b in range(2)]
        hsum = [sb(f"hsum{b}", [R, 1], f32) for b in range(2)]

        scr = ctx.enter_context(tc.tile_pool(name="scr", bufs=4))
        treep = ctx.enter_context(tc.tile_pool(name="treep", bufs=2))
        term_pool = ctx.enter_context(tc.tile_pool(name="terms", bufs=3))
        osb_pool = ctx.enter_context(tc.tile_pool(name="osbp", bufs=8))
        ct_pool = ctx.enter_context(tc.tile_pool(name="ctp", bufs=3))
        psA = ctx.enter_context(tc.tile_pool(name="psA", bufs=1, space="PSUM"))
        psB = ctx.enter_context(tc.tile_pool(name="psB", bufs=1, space="PSUM"))
        # stats bank: single-shot matmul groups only may share a bank
        stpa = nc.alloc_psum_tensor("statps", [128, 512], f32).ap()

        # ---------------- DMA emission ----------------
        row_chunks = [(0, 33), (33, 65), (65, HP)]

        def emit_xA_dma(g):
            for (r0, r1) in row_chunks:
                nc.sync.dma_start(
                    out=padA[g][:, r0:r1, :],
                    in_=xA_d[g * 128:(g + 1) * 128, r0:r1, :],
                )

        def emit_xB_dma(g):
            # WAW gate: tiny DVE write into padB dependent on g1's last xA
            # chunk keeps the xB transfer out of the critical input window
            nc.vector.tensor_copy(padB[g][:, 0:1, 0:2], padA[1][:, 97:98, 0:2])
            nc.scalar.dma_start(
                out=padB[g][:, :, :],
                in_=xB_d[g * 128:(g + 1) * 128, :, :],
            )

        emit_xA_dma(0)
        wloads = [
            (eye, eye_d), (w1avg_a, w1avg_a_d), (w1avg_b, w1avg_b_d),
            (w1mx_a, w1mx_a_d), (w1mx_b, w1mx_b_d), (w2t, w2t_d),
            (p1a, p1a_d), (p1b, p1b_d), (bns, bns_d), (bnb, bnb_d),
            (w2s, w2s_d),
            (adkT[0], adkT_d[0:128, :]), (adkT[1], adkT_d[128:256, :]),
            (adkT[2], adkT_d[256:384, :]),
        ]
        for (dst, src) in wloads:
            nc.gpsimd.dma_start(out=dst, in_=src)
        emit_xA_dma(2)
        emit_xA_dma(1)

        # ---------------- stats ----------------
        def emit_band_windows(g, k, engine="vector"):
            # 3 col-window sums of the 32-row band k -> pooled[g][:, 3k+j]
            if engine == "scalar":
                for j in range(3):
                    win = padA[g][:, 1 + 32 * k:33 + 32 * k, 2 + 32 * j:34 + 32 * j]
                    acc = pooled[g][:, 3 * k + j:3 * k + j + 1]
                    s = treep.tile([128, 32, 32], bf16, tag="wscr", name=f"w{g}_{k}_{j}")
                    nc.scalar.activation(out=s[:, :, :], in_=win,
                                         func=ACTF.Copy, accum_out=acc)
                return
            # DVE: bf16 TT add-tree 32->16->8->4 rows (2x mode), then 3
            # cache-reduce window sums over the 4 leaf rows
            p = padA[g]
            r0 = 1 + 32 * k
            t16 = treep.tile([128, 16, WP], bf16, tag="tr16", name=f"s16_{g}_{k}")
            nc.vector.tensor_add(t16[:, :, :], p[:, r0:r0 + 16, :], p[:, r0 + 16:r0 + 32, :])
            t8 = treep.tile([128, 8, WP], bf16, tag="tr8", name=f"s8_{g}_{k}")
            nc.vector.tensor_add(t8[:, :, :], t16[:, 0:8, :], t16[:, 8:16, :])
            t4 = treep.tile([128, 4, WP], bf16, tag="tr4", name=f"s4_{g}_{k}")
            nc.vector.tensor_add(t4[:, :, :], t8[:, 0:4, :], t8[:, 4:8, :])
            for j in range(3):
                acc = pooled[g][:, 3 * k + j:3 * k + j + 1]
                s = treep.tile([128, 4, 32], bf16, tag="wscr4", name=f"w{g}_{k}_{j}")
                nc.vector.tensor_scalar(s[:, :, :],
                                        t4[:, :, 2 + 32 * j:34 + 32 * j],
                                        1.0, None,
                                        op0=ALU.mult, op1=ALU.add, accum_out=acc)

        lvmax = [sb(f"lvmax{g}", [128, 12, WP], bf16) for g in range(3)]

        def emit_band_max(g, k):
            # bf16 TT max-tree 32->16->8->4 rows into lvmax[g][:, 4k:4k+4]
            p = padA[g]
            r0 = 1 + 32 * k
            t16 = treep.tile([128, 16, WP], bf16, tag="tr16", name=f"m16_{g}_{k}")
            nc.vector.tensor_tensor(out=t16[:, :, :], in0=p[:, r0:r0 + 16, :],
                                    in1=p[:, r0 + 16:r0 + 32, :], op=ALU.max)
            t8 = treep.tile([128, 8, WP], bf16, tag="tr8", name=f"m8_{g}_{k}")
            nc.vector.tensor_tensor(out=t8[:, :, :], in0=t16[:, 0:8, :],
                                    in1=t16[:, 8:16, :], op=ALU.max)
            nc.vector.tensor_tensor(out=lvmax[g][:, 4 * k:4 * k + 4, :],
                                    in0=t8[:, 0:4, :], in1=t8[:, 4:8, :], op=ALU.max)

        def emit_mx_fin(g):
            t6 = treep.tile([128, 6, WP], bf16, tag="tr6", name=f"mf6_{g}")
            nc.vector.tensor_tensor(out=t6[:, :, :], in0=lvmax[g][:, 0:6, :],
                                    in1=lvmax[g][:, 6:12, :], op=ALU.max)
            t3 = treep.tile([128, 3, WP], bf16, tag="tr3", name=f"mf3_{g}")
            nc.vector.tensor_tensor(out=t3[:, :, :], in0=t6[:, 0:3, :],
                                    in1=t6[:, 3:6, :], op=ALU.max)
            nc.vector.tensor_reduce(out=mx[g][:, :], in_=t3[:, :, :],
                                    axis=mybir.AxisListType.XY, op=ALU.max)

        def emit_avg_fin(g):
            asc = scr.tile([128, 9], bf16, tag="ascr", name=f"avg{g}")
            nc.scalar.activation(out=asc[:, :], in_=pooled[g][:, :],
                                 func=ACTF.Copy, accum_out=avgs[g][:, :])

        # ---------------- per-sample algebra ----------------
        def emit_sample(b, part):
            if b == 0:
                chunks = [
                    (w1avg_a[:, :], w1mx_a[:, :], p1a[:, :], (0, 0, 128)),
                    (w1avg_b[0:64, :], w1mx_b[0:64, :], p1b[0:64, :], (2, 0, 64)),
                ]
            else:
                chunks = [
                    (w1avg_a[:, :], w1mx_a[:, :], p1a[:, :], (1, 0, 128)),
                    (w1avg_b[64:128, :], w1mx_b[64:128, :], p1b[64:128, :], (2, 64, 128)),
                ]
            base = 8 + b * 22
            for i, (wa, wm, wp, (sg, q0, q1)) in enumerate(chunks):
                o = base + 11 * i
                if part == "pool":
                    nc.tensor.matmul(stpa[0:R, o + 2:o + 11], lhsT=wp, rhs=pooled[sg][q0:q1, :], start=True, stop=True)
                else:
                    nc.tensor.matmul(stpa[0:R, o:o + 1], lhsT=wa, rhs=avgs[sg][q0:q1, :], start=True, stop=True)
                    nc.tensor.matmul(stpa[0:R, o + 1:o + 2], lhsT=wm, rhs=mx[sg][q0:q1, :], start=True, stop=True)

        def emit_fold_pool(b):
            base = 8 + b * 22
            hc = scr.tile([R, 9], f32, tag="scr48", name=f"hc{b}")
            nc.vector.tensor_copy(hc[:, :], stpa[0:R, base + 2:base + 11])
            hs = scr.tile([R, 9], f32, tag="scr48", name=f"hs{b}")
            nc.vector.tensor_add(hs[:, :], hc[:, :],
                                 stpa[0:R, base + 13:base + 22])
            t1 = scr.tile([R, 9], f32, tag="scr48", name=f"bn{b}")
            nc.vector.tensor_scalar(t1[:, :], hs[:, :], bns[:, :], bnb[:, :],
                                    op0=ALU.mult, op1=ALU.add)
            nc.vector.tensor_scalar_max(h_adk[b][:, :], t1[:, :], 0.0)

        def emit_fold_theta(b):
            base = 8 + b * 22
            hg = scr.tile([R, 2], f32, tag="scr2", name=f"hg{b}")
            nc.vector.tensor_copy(hg[:, :], stpa[0:R, base:base + 2])
            hs = scr.tile([R, 2], f32, tag="scr2", name=f"ht{b}")
            nc.vector.tensor_add(hs[:, :], hg[:, :],
                                 stpa[0:R, base + 11:base + 13])
            ha = scr.tile([R, 1], f32, tag="scr1", name=f"ha{b}")
            hm = scr.tile([R, 1], f32, tag="scr1", name=f"hm{b}")
            nc.vector.tensor_scalar_max(ha[:, :], hs[:, 0:1], 0.0)
            nc.vector.tensor_scalar_max(hm[:, :], hs[:, 1:2], 0.0)
            nc.vector.tensor_add(hsum[b][:, :], ha[:, :], hm[:, :])

        # ---------------- theta ----------------
        ps_t = [stpa[:, i:i + 1] for i in range(3)]

        def emit_theta_mm(b):
            nc.tensor.matmul(ps_t[b], lhsT=w2t[:, 0:128], rhs=hsum[b][:, :], start=True, stop=True)
            q0, q1 = (0, 64) if b == 0 else (64, 128)
            nc.tensor.matmul(ps_t[2][q0:q1], lhsT=w2t[:, 128:192], rhs=hsum[b][:, :], start=True, stop=True)

        def emit_theta_fin(g):
            et = scr.tile([128, 1], f32, tag="scr1", name=f"et{g}")
            nc.scalar.activation(out=et[:, :], in_=ps_t[g], func=ACTF.Exp, scale=-1.0)
            d = scr.tile([128, 1], f32, tag="scr1", name=f"etd{g}")
            nc.vector.tensor_scalar_add(d[:, :], et[:, :], 1.0)
            nc.vector.reciprocal(th[g][:, :], d[:, :])

        # ---------------- dynamic kernels w9 ----------------
        def emit_w9_mm(g):
            ps_s = stpa[:, 64 + g * 36:64 + (g + 1) * 36]
            for gg in range(G):
                sl = slice(gg * 9, gg * 9 + 9)
                if g < 2:
                    nc.tensor.matmul(ps_s[:, sl], lhsT=w2s[:, gg * 192:gg * 192 + 128],
                                     rhs=h_adk[g][:, :], start=True, stop=True)
                else:
                    nc.tensor.matmul(ps_s[0:64, sl], lhsT=w2s[:, gg * 192 + 128:gg * 192 + 192],
                                     rhs=h_adk[0][:, :], start=True, stop=True)
                    nc.tensor.matmul(ps_s[64:128, sl], lhsT=w2s[:, gg * 192 + 128:gg * 192 + 192],
                                     rhs=h_adk[1][:, :], start=True, stop=True)

        def emit_w9_exp(g):
            ps_s = stpa[:, 64 + g * 36:64 + (g + 1) * 36]
            e = scr.tile([128, 36], f32, tag="scr36", name=f"e{g}")
            nc.scalar.activation(out=e[:, :], in_=ps_s, func=ACTF.Exp)
            return e

        def emit_w9_fin(g, e):
            d1 = scr.tile([128, 9], f32, tag="scr9", name=f"d1_{g}")
            d2 = scr.tile([128, 9], f32, tag="scr9", name=f"d2_{g}")
            nc.vector.tensor_add(d1[:, :], e[:, 0:9], e[:, 9:18])
            nc.vector.tensor_add(d2[:, :], e[:, 18:27], e[:, 27:36])
            nc.vector.tensor_add(d1[:, :], d1[:, :], d2[:, :])
            rec = scr.tile([128, 9], f32, tag="scr9", name=f"rec{g}")
            nc.vector.reciprocal(rec[:, :], d1[:, :])
            a = adkT[g]
            m1 = scr.tile([128, 9], f32, tag="scr9", name=f"m1_{g}")
            m2 = scr.tile([128, 9], f32, tag="scr9", name=f"m2_{g}")
            nc.vector.tensor_mul(m1[:, :], e[:, 0:9], a[:, 0:9])
            nc.vector.tensor_mul(m2[:, :], e[:, 9:18], a[:, 9:18])
            nc.vector.tensor_add(m1[:, :], m1[:, :], m2[:, :])
            nc.vector.tensor_mul(m2[:, :], e[:, 18:27], a[:, 18:27])
            nc.vector.tensor_add(m1[:, :], m1[:, :], m2[:, :])
            nc.vector.tensor_mul(m2[:, :], e[:, 27:36], a[:, 27:36])
            nc.vector.tensor_add(m1[:, :], m1[:, :], m2[:, :])
            nc.vector.tensor_mul(w9[g][:, :], m1[:, :], rec[:, :])
            nc.vector.tensor_reduce(out=wsum9[g][:, :], in_=w9[g][:, :],
                                    axis=mybir.AxisListType.X, op=ALU.add)

        def emit_w4p(g):
            t1 = scr.tile([128, 1], f32, tag="scr1", name=f"t1_{g}")
            nc.vector.tensor_mul(t1[:, :], w9[g][:, 4:5], th[g][:, :])
            nc.vector.tensor_add(t1[:, :], t1[:, :], w9[g][:, 4:5])
            nc.vector.tensor_sub(w4p[g][:, :], t1[:, :], wsum9[g][:, :])

        def emit_diag(g, taps, engine):
            for tap in taps:
                scal = w4p[g][:, 0:1] if tap == 4 else w9[g][:, tap:tap + 1]
                if engine == "vector":
                    nc.vector.tensor_scalar_mul(diag[g][:, tap, :], eye[:, :], scal)
                else:
                    nc.scalar.activation(out=diag[g][:, tap, :], in_=eye[:, :],
                                         func=ACTF.Copy, scale=scal)

        # ---------------- conv on PE ----------------
        # window w of group g covers rows win_r0 .. win_r0+5*banks
        win_r0 = {}
        for g in range(3):
            r = 0
            for w, nb in enumerate(WIN_SEQ[g]):
                win_r0[(g, w)] = r
                r += 5 * nb

        pools = {4: psA, 3: psB}
        win_tile = {}
        out_rr = [0]

        def conv_taps(g, w, taps):
            nb = WIN_SEQ[g][w]
            r0 = win_r0[(g, w)]
            key = (g, w)
            if key not in win_tile:
                pnb = 4 if w % 2 == 0 else 3
                win_tile[key] = pools[pnb].tile(
                    [128, pnb, 512], f32, tag=f"w{pnb}", name=f"ps{g}_{w}")
            ps = win_tile[key]
            for tap in taps:
                dy, dx = divmod(tap, 3)
                for b in range(nb):
                    y0 = r0 + 5 * b + dy
                    nc.tensor.matmul(
                        ps[:, b, 0:480],
                        lhsT=diag[g][:, tap, :],
                        rhs=padA[g][:, y0:y0 + 5, dx + 1:dx + 97],
                        start=(tap == taps[0]), stop=(tap == taps[-1]),
                    )

        def conv_drain(g, w):
            nb = WIN_SEQ[g][w]
            r0 = win_r0[(g, w)]
            ps = win_tile.pop((g, w))
            nr = 5 * nb
            ot = osb_pool.tile([128, 20, W], bf16, tag="ow", name=f"ow{g}_{w}")
            nc.scalar.activation(
                out=ot[:, 0:nr, :],
                in_=ps[:, 0:nb, 0:480], func=ACTF.Copy)
            eng = (nc.sync, nc.gpsimd)[out_rr[0] % 2]
            out_rr[0] += 1
            eng.dma_start(
                out=out_d[g * 128:(g + 1) * 128, r0:r0 + nr, :],
                in_=ot[:, 0:nr, :])

        # ---------------- conv tail on DVE ----------------
        def emit_conv_dve(g, lo, hi):
            # DVE tail rows: flat contiguous strips (full padded rows, junk
            # at pad columns) so the muls hit the 4x DVE mode; only the
            # final add back into tailb is strided.
            y0 = ROWS_PE[g] + lo
            n = hi - lo
            L = (n - 1) * 100 + 96
            acc = None
            for i, tap in enumerate(TAP_ORDER):
                dy, dx = divmod(tap, 3)
                scal = w4p[g][:, 0:1] if tap == 4 else w9[g][:, tap:tap + 1]
                if dx == 1:
                    o0 = (y0 + dy) * 100 + 2
                    strip = padAf[g][:, o0:o0 + L]
                else:
                    o0 = (y0 + dy - XB_R0) * 100 + (2 if dx == 0 else 4)
                    strip = padBf[g][:, o0:o0 + L]
                t = term_pool.tile([128, 14, 100], bf16, tag="term",
                                   name=f"t{g}_{lo}_{i}")
                tf = t.rearrange("p a b -> p (a b)")
                nc.vector.tensor_scalar_mul(tf[:, 0:L], strip, scal)
                if i == 0:
                    acc = t
                elif i < 8:
                    nxt = term_pool.tile([128, 14, 100], bf16, tag="term",
                                         name=f"a{g}_{lo}_{i}")
                    nxf = nxt.rearrange("p a b -> p (a b)")
                    nc.vector.tensor_add(nxf[:, 0:L],
                                         acc.rearrange("p a b -> p (a b)")[:, 0:L],
                                         tf[:, 0:L])
                    acc = nxt
                else:
                    nc.vector.tensor_add(tailb[g][:, lo:hi, :],
                                         acc[:, 0:n, 0:96], t[:, 0:n, 0:96])
            eng = (nc.sync, nc.gpsimd)[out_rr[0] % 2]
            out_rr[0] += 1
            eng.dma_start(out=out_d[g * 128:(g + 1) * 128, y0:y0 + n, :],
                          in_=tailb[g][:, lo:hi, :])

        # ---------------- PE warm-up ----------------
        # junk matmuls gated on successive DMA chunks so the PE stays busy
        # (HAM warm) across the whole stats prelude without running eagerly
        def emit_warmup(k, gate, drain=False):
            for j in range(k):
                nc.tensor.matmul(stpa[:, 384:512], lhsT=eye[:, :],
                                 rhs=gate, start=True, stop=True)
            if drain:
                wsc = scr.tile([128, 1], bf16, tag="wscr1", name="wscr")
                nc.scalar.activation(out=wsc[:, :], in_=stpa[:, 384:385], func=ACTF.Copy)
                nc.sync.dma_start(out=warm_d, in_=wsc[:, :])

        # ---------------- emission order ----------------
        NC8 = TAP_ORDER[:8]

        # prelude: window sums first (pooled -> w9 -> conv is the critical
        # path; image-max/theta only gates the post-drain center-tap add)
        for k in range(3):
            emit_band_windows(0, k, engine="vector")
        emit_band_windows(2, 0, engine="scalar")
        emit_band_windows(2, 1, engine="scalar")
        emit_band_windows(2, 2, engine="vector")
        emit_avg_fin(0)
        emit_avg_fin(2)
        emit_xB_dma(0)
        emit_xB_dma(1)
        emit_xB_dma(2)
        emit_warmup(30, padA[0][:, 65:67, 0:64])
        emit_warmup(30, padA[2][:, 65:67, 0:64])
        emit_sample(0, "pool")
        emit_fold_pool(0)
        emit_w9_mm(0)
        e0 = emit_w9_exp(0)
        emit_w9_fin(0, e0)
        emit_diag(0, NC8, "vector")
        for k in range(3):
            emit_band_max(0, k)
            emit_band_max(2, k)
        emit_mx_fin(0)
        emit_mx_fin(2)
        emit_warmup(24, padA[1][:, 10:12, 0:64])
        emit_warmup(16, padA[1][:, 40:42, 0:64], drain=True)
        emit_sample(0, "theta")
        emit_fold_theta(0)
        emit_theta_mm(0)
        emit_theta_fin(0)
        emit_w4p(0)
        emit_diag(0, [4], "vector")

        conv_taps(0, 0, TAP_ORDER)
        conv_taps(0, 1, TAP_ORDER)

        # g1 stats (data lands mid-conv-g0); sums on ScalarE (slack there)
        for k in range(3):
            emit_band_windows(1, k, engine="scalar")
            emit_band_max(1, k)
        emit_mx_fin(1)
        emit_avg_fin(1)

        conv_drain(0, 0)
        conv_taps(0, 2, TAP_ORDER)
        emit_sample(1, "pool")
        emit_fold_pool(1)
        emit_sample(1, "theta")
        emit_fold_theta(1)
        emit_theta_mm(1)
        emit_theta_fin(1)
        emit_w9_mm(1)
        e1 = emit_w9_exp(1)
        emit_w9_fin(1, e1)
        emit_w4p(1)
        emit_diag(1, TAP_ORDER, "scalar")
        conv_drain(0, 1)
        conv_taps(0, 3, TAP_ORDER)
        emit_theta_fin(2)
        emit_w9_mm(2)
        e2 = emit_w9_exp(2)
        emit_w9_fin(2, e2)
        emit_w4p(2)
        emit_diag(2, TAP_ORDER, "scalar")
        conv_drain(0, 2)
        conv_taps(1, 0, TAP_ORDER)
        conv_drain(0, 3)
        emit_conv_dve(0, 0, 13)
        conv_taps(1, 1, TAP_ORDER)
        conv_drain(1, 0)
        emit_conv_dve(0, 13, 26)
        conv_taps(1, 2, TAP_ORDER)
        conv_drain(1, 1)
        emit_conv_dve(1, 0, 13)
        conv_taps(1, 3, TAP_ORDER)
        conv_drain(1, 2)
        emit_conv_dve(1, 13, 26)
        conv_taps(2, 0, TAP_ORDER)
        conv_drain(1, 3)
        emit_conv_dve(1, 26, 31)
        conv_taps(2, 1, TAP_ORDER)
        conv_drain(2, 0)
        emit_conv_dve(2, 0, 13)
        conv_taps(2, 2, TAP_ORDER)
        conv_drain(2, 1)
        emit_conv_dve(2, 13, 26)
        conv_taps(2, 3, TAP_ORDER)
        conv_drain(2, 2)
        conv_drain(2, 3)

    nc.compile()
    return nc


def _host_prep(inputs):
    x = np.ascontiguousarray(inputs["x"], dtype=np.float32)
    cam_w1 = np.asarray(inputs["cam_w1"], dtype=np.float32)
    cam_w2 = np.asarray(inputs["cam_w2"], dtype=np.float32)
    proj_w1 = np.asarray(inputs["proj_w1"], dtype=np.float32)
    bn_gamma = np.asarray(inputs["bn_gamma"], dtype=np.float32)
    bn_beta = np.asarray(inputs["bn_beta"], dtype=np.float32)
    proj_w2 = np.asarray(inputs["proj_w2"], dtype=np.float32)
    adk = np.asarray(inputs["adk_weight"], dtype=np.float32)

    xb16 = x.astype(BF16)
    xpA = np.zeros((B, C, HP, WP), dtype=BF16)
    xpA[:, :, 1:97, 2:98] = xb16
    # padB: x payload at col 3, rows = padded rows 50..97 (x rows 49..95)
    xpB = np.zeros((B, C, XB_NR, WP), dtype=BF16)
    xpB[:, :, 0:47, 3:99] = xb16[:, :, 49:96, :]

    in_maps = []
    w1t = cam_w1.T.astype(np.float32)
    p1t = (proj_w1.T / 1024.0).astype(np.float32)
    cmap = np.concatenate([np.arange(128), np.arange(128),
                           np.arange(128, 192), np.arange(128, 192)])
    consts = {
        "eye": np.eye(128, dtype=BF16),
        "w1avg_a": np.ascontiguousarray(w1t[0:128] / (H * W)),
        "w1avg_b": np.ascontiguousarray(np.concatenate([w1t[128:192] / (H * W)] * 2, axis=0)),
        "w1mx_a": np.ascontiguousarray(w1t[0:128]),
        "w1mx_b": np.ascontiguousarray(np.concatenate([w1t[128:192]] * 2, axis=0)),
        "w2t": np.ascontiguousarray(cam_w2.T.astype(np.float32)),
        "p1a": np.ascontiguousarray(p1t[0:128]),
        "p1b": np.ascontiguousarray(np.concatenate([p1t[128:192]] * 2, axis=0)),
        "bn_scale": np.ascontiguousarray((bn_gamma / np.sqrt(1.0 + BN_EPS)).reshape(R, 1)),
        "bn_beta": np.ascontiguousarray(bn_beta.reshape(R, 1)),
        "w2s": np.ascontiguousarray(proj_w2.T.astype(np.float32)),
        "adkT": np.ascontiguousarray(
            adk.transpose(1, 0, 2, 3).reshape(C, G * 9)[cmap].astype(np.float32)
        ),
    }
    for k in range(N_CORES):
        b0, b1 = 2 * k, 2 * k + 1
        shardA = np.ascontiguousarray(np.concatenate(
            [xpA[b0, 0:128], xpA[b1, 0:128], xpA[b0, 128:192], xpA[b1, 128:192]],
            axis=0))
        shardB = np.ascontiguousarray(np.concatenate(
            [xpB[b0, 0:128], xpB[b1, 0:128], xpB[b0, 128:192], xpB[b1, 128:192]],
            axis=0))
        m = {"xA": shardA, "xB": shardB}
        m.update(consts)
        in_maps.append(m)
    return in_maps


def kernel(**inputs) -> np.ndarray:
    global _COMPILED
    from concourse.bass_utils import run_bass_kernel_spmd

    in_maps = _host_prep(inputs)

    if _COMPILED is None:
        _COMPILED = _build()
    nc = _COMPILED

    res = run_bass_kernel_spmd(nc, in_maps, core_ids=list(range(N_CORES)))
    outs = [r["out"] for r in res.results]

    y = np.empty((B, C, H, W), np.float32)
    for k in range(N_CORES):
        o = np.asarray(outs[k]).reshape(384, H, W).astype(np.float32)
        b0, b1 = 2 * k, 2 * k + 1
        y[b0, 0:128] = o[0:128]
        y[b1, 0:128] = o[128:256]
        y[b0, 128:192] = o[256:320]
        y[b1, 128:192] = o[320:384]
    return y


if __name__ == "__main__":
    import reference

    inputs = {k: np.asarray(v) for k, v in reference.setup_inputs().items()}
    y = kernel(**inputs)
    print("kernel output:", y.shape, y.dtype)

